# revision 35
# baseline (speedup 1.0000x reference)
"""Trainium2 Bass kernel for nn_AutoCorr2D.

Computation (per sample):
  f   = conv3x3(x, w_ext, pad=1) + b_ext            # [CC=128, 64, 64]
  corr[c,i,j,k] = f[c,i,j] * fpad[c, i+u-2, j+v-2]  # 5x5 window products
  out[o,i,j]    = sum_{c,k} w_reg[o,c,k] * corr[c,i,j,k] + b_reg[o]

Sharding: data-parallel over batch B=8 across 8 NeuronCores (one sample per
core); conv weights replicated.

Per-core implementation (all matmul operands bf16, host-pre-cast):
  input: x / w_ext / w_reg are cast to bf16 on the host, halving HBM
         traffic; x is DMA'd in two >0.5MB strided transfers per cin tile
         (large transfers split across all 16 SDMA engines; the banded
         small-DMA + engine-cast pipeline measured ~3x slower and paced the
         whole conv) straight into the zero-bordered xpad layout, split
         between the Sync and GpSimd HWDGE queues; weights go on ScalarE's
         queue.
  stage 1: implicit GEMM over (cin_tile, 3x3 tap): 18 accumulating bf16
           matmuls per 512-pixel chunk reading shifted xpad views; the
           PSUM->SBUF copy (ScalarE, bias folded) writes the features
           twice: fpad and fpad_odd (the same features shifted one flat
           element, so odd-column-shift taps read 4B-aligned bf16 pairs).
  stage 2: product symmetry: P_{a,b}[y,x] = f[y,x]*f[y+a,x+b] serves both
           tap (a,b) and (-a,-b) via shifted reads, so only 13 of 25
           product maps exist (ScalarE Square for (0,0); VectorE bf16
           tensor_mul at 2 elem/cycle for the rest).  The regressor GEMM
           packs the M=64 matmuls pairwise into the PE's column-tiled
           128x64 mode (tile_position (0,0)/(0,64), separate PSUM banks):
           the two array halves stream different taps concurrently.
           ScalarE evacuates both PSUM halves promptly (+b_reg on one);
           VectorE adds the two SBUF tensors (its FIFO is deep in product
           maps, so it must not gate PSUM reuse); DMA out on Sync.
  The PE is pre-warmed with dummy matmuls so the HAM clock gate releases
  before real work; mid-kernel PE gaps >3.4us re-throttle it (avoided by
  keeping the x feed and product maps ahead of the PE).
"""

import ml_dtypes
import numpy as np

from concourse import bacc, mybir, tile
from concourse.bass_utils import run_bass_kernel_spmd

B, CIN, H, W = 8, 256, 64, 64
CC, COUT = 128, 64
HW = H * W
NCORES = 8

NCHUNK = 8           # pixel chunks per image
CROWS = H // NCHUNK  # rows per chunk (8) -> N = 512 pixels
NPX = CROWS * W      # 512
NGRP = 4             # product-map groups (2 chunks each)
GROWS = 2 * CROWS    # 16

XP = W + 4           # xpad cols (pad=1 + one spare col so the interior
                     # starts at an even element: packed 16-bit DVE writes
                     # need 4B alignment)
XR = H + 2           # xpad rows
FP = W + 4           # fpad cols (pad=2)
FR = H + 4           # fpad rows
FTAIL = 72           # guard tail so shifted product reads stay in-bounds

# The 13 "upper half" taps; (a,b) also serves tap (-a,-b) via a shifted read.
SYM = [(0, 0), (0, 1), (0, 2),
       (1, -2), (1, -1), (1, 0), (1, 1), (1, 2),
       (2, -2), (2, -1), (2, 0), (2, 1), (2, 2)]

F32 = mybir.dt.float32
F32R = mybir.dt.float32r
BF16 = mybir.dt.bfloat16
U32 = mybir.dt.uint32
U16 = mybir.dt.uint16
AF = mybir.ActivationFunctionType


def build_body(nc, tc, x, wext, wreg, bext, breg, out):
    with (
        tc.tile_pool(name="const", bufs=1) as constp,
        tc.tile_pool(name="xpadp", bufs=1) as xpadp,
        tc.tile_pool(name="fpadp", bufs=1) as fpadp,
        tc.tile_pool(name="prodp", bufs=4) as prodp,
        tc.tile_pool(name="outp", bufs=2) as outp,
        tc.tile_pool(name="ps1", bufs=3, space="PSUM") as ps1,
        tc.tile_pool(name="ps2a", bufs=3, space="PSUM") as ps2a,
        tc.tile_pool(name="ps2b", bufs=2, space="PSUM") as ps2b,
    ):
        # PE warm-up: dummy matmuls on a zeroed f32r scratch start immediately
        # and release the HAM clock gate (~3.4us of activity) before real
        # matmuls begin; sized so the last one ends right as chunk 0's data
        # lands (a >3.4us PE gap here would re-throttle and run chunk 0 at
        # half clock).  The warm-up PSUM shares the ps1 pool (banks:
        # 3 + 3 + 2 = 8).
        wsc_r = constp.tile([128, NPX], F32R, name="wsc_r")
        nc.vector.memset(wsc_r.bitcast(U32), 0)
        # 10 mms (~4.6us): the HAM flips to full clock DURING the warm-up
        # (measured ~3.2-3.4us in); the short idle gap before conv chunk 0's
        # data lands then stays warm.  Shorter warm-ups leave the flip until
        # minutes into chunk 0, which then runs at half clock.
        wpsum = ps1.tile([128, NPX], F32, name="wpsum", tag="psum1")
        for i in range(10):
            nc.tensor.matmul(wpsum, wsc_r[:, :128], wsc_r,
                             start=(i == 0), stop=(i == 9))

        # ---- x: bf16.  One big contiguous DMA per cin tile (large DMAs
        # split across all 16 SDMA engines at ~340GB/s; strided or banded
        # small DMAs measured 3-7x slower and gated the conv), queue per
        # cin tile (Sync / GpSimd), then wide packed-bf16 pad-scatter
        # copies on DVE into the zero-bordered xpad layout.
        xpads = []
        xsts = []
        for t in range(2):
            xp = xpadp.tile([128, XR * XP], BF16, name=f"xpad{t}",
                            tag=f"xpad{t}")
            xr = xp.rearrange("p (r c) -> p r c", c=XP)
            xri = xp.bitcast(U16).rearrange("p (r c) -> p r c", c=XP)
            nc.vector.memset(xri[:, 0, :], 0)
            nc.vector.memset(xri[:, XR - 1, :], 0)
            nc.vector.memset(xri[:, 1:XR - 1, 0:2], 0)
            nc.vector.memset(xri[:, 1:XR - 1, XP - 2:XP], 0)
            xpads.append(xr)
            xst = constp.tile([128, HW], BF16, name=f"xst{t}")
            # x on the Scalar/GpSimd queues; the Sync queue carries the
            # (smaller) weights so they don't queue behind 2MB of x and
            # stall chunk 0's LDWEIGHTS for several us
            q = nc.scalar if t == 0 else nc.gpsimd
            # three bands per stream: a tiny 10-row first band lands in
            # ~1us even while 3MB of weights+x are in flight, so conv
            # chunk 0 starts as soon as the warm-up ends
            for rlo, rhi in ((0, 10), (10, 32), (32, 64)):
                q.dma_start(out=xst[:, rlo * W:rhi * W],
                            in_=x[t * 128:(t + 1) * 128, rlo * W:rhi * W])
            xsts.append(xst)
        # matching scatter bands (chunk i waits only on the bands that
        # cover its rows)
        for rlo, rhi in ((0, 10), (10, 32), (32, 64)):
            for t in range(2):
                stv = xsts[t][:, rlo * W:rhi * W].rearrange(
                    "p (r c) -> p r c", c=W)
                nc.vector.tensor_copy(
                    xpads[t][:, 1 + rlo:1 + rhi, 2:2 + W], stv)

        bext_sb = constp.tile([128, 1], F32, name="bext_sb")
        breg_sb = constp.tile([64, 1], F32, name="breg_sb")

        # Weights (host-pre-cast bf16) first on the Sync HWDGE queue so
        # they land before the x bulk; wext split so the first 9 lhsT
        # blocks (cin tile 0) land earliest.
        wext_sb = constp.tile([128, 18 * 128], BF16, name="wext_sb")
        wreg_sb = constp.tile([128, 25 * 64], BF16, name="wreg_sb")
        WSPLIT = ((0, 3), (3, 9), (9, 18))
        for lo, hi in WSPLIT:
            nc.sync.dma_start(out=wext_sb[:, lo * 128:hi * 128],
                              in_=wext[:, lo * 128:hi * 128])
        nc.sync.dma_start(out=bext_sb, in_=bext)
        nc.sync.dma_start(out=breg_sb, in_=breg)
        nc.sync.dma_start(out=wreg_sb, in_=wreg)

        # ---- padded features (pad=2, bf16) + guard tail; fpad_odd is
        # fpad shifted one flat element earlier (odd[k] == fpad[k+1])
        # so taps with odd column shift read 4B-aligned bf16 pairs ----
        fpad = fpadp.tile([128, FR * FP + FTAIL], BF16, name="fpad")
        fodd = fpadp.tile([128, FR * FP + FTAIL], BF16, name="fodd")
        fr = fpad[:, :FR * FP].rearrange("p (r c) -> p r c", c=FP)
        fo = fodd[:, :FR * FP].rearrange("p (r c) -> p r c", c=FP)
        fpi = fpad.bitcast(U16)
        fri = fpi[:, :FR * FP].rearrange("p (r c) -> p r c", c=FP)
        foi_full = fodd.bitcast(U16)
        foi = foi_full[:, :FR * FP].rearrange("p (r c) -> p r c", c=FP)
        # fpad borders: rows 0-1, bottom rows + tail, col pads; fodd gets
        # the -1-shifted mirror
        nc.vector.memset(fpi[:, 0:2 * FP], 0)
        nc.vector.memset(fpi[:, (FR - 2) * FP:FR * FP + FTAIL], 0)
        nc.vector.memset(fri[:, 2:FR - 2, 0:2], 0)
        nc.vector.memset(fri[:, 2:FR - 2, FP - 2:FP], 0)
        nc.vector.memset(foi_full[:, 0:2 * FP], 0)
        nc.vector.memset(foi_full[:, (FR - 2) * FP - 1:FR * FP + FTAIL], 0)
        nc.vector.memset(foi[:, 2:FR - 2, 0], 0)
        nc.vector.memset(foi[:, 2:FR - 2, FP - 3:FP], 0)

        prodtiles = [[None] * len(SYM) for _ in range(NGRP)]

        def make_square(g):
            # tap (0,0) product map on ScalarE (Square), bf16
            base = (g * GROWS + 2) * FP
            pt = prodp.tile([128, GROWS * FP], BF16, name=f"sq{g}",
                            tag="prod0", bufs=4)
            nc.scalar.activation(pt, fpad[:, base:base + GROWS * FP],
                                 AF.Square)
            prodtiles[g][0] = pt

        # ---- stage 1: f = conv3x3(x) + b_ext ----
        for i in range(NCHUNK):
            psum1 = ps1.tile([128, NPX], F32, name="psum1", tag="psum1")
            k = 0
            for t in range(2):
                for du in range(3):
                    for dv in range(3):
                        rhs = xpads[t][:,
                                       i * CROWS + du:
                                       i * CROWS + du + CROWS,
                                       1 + dv:1 + dv + W]
                        blk = t * 9 + du * 3 + dv
                        lhsT = wext_sb[:, blk * 128:(blk + 1) * 128]
                        nc.tensor.matmul(psum1, lhsT, rhs,
                                         start=(k == 0), stop=(k == 17))
                        k += 1
            pv = psum1.rearrange("p (r c) -> p r c", c=W)
            dst = fr[:, i * CROWS + 2:i * CROWS + 2 + CROWS, 2:2 + W]
            nc.scalar.activation(dst, pv, AF.Identity,
                                 bias=bext_sb, scale=1.0)
            dsto = fo[:, i * CROWS + 2:i * CROWS + 2 + CROWS, 1:1 + W]
            nc.scalar.activation(dsto, pv, AF.Identity,
                                 bias=bext_sb, scale=1.0)
            # squares as soon as their 2-chunk group is complete
            if i % 2 == 1:
                make_square(i // 2)

        # ---- stage 2a: all product maps (VectorE, bf16 2x/cycle) ----
        # Emitted for all groups before any finish-add so the DVE FIFO
        # never head-of-line blocks products behind an op that waits on
        # the PE.  bufs=4: all four groups live at once, so no group WARs
        # on GEMM consumers.
        for g in range(NGRP):
            for kk, (a, b) in enumerate(SYM):
                if kk == 0:
                    continue
                nrows = GROWS + a
                base = (g * GROWS + 2 - a) * FP
                pt = prodp.tile([128, nrows * FP], BF16,
                                name=f"prod{g}_{kk}", tag=f"prod{kk}",
                                bufs=4)
                in0 = fpad[:, base:base + nrows * FP]
                off = base + a * FP + b
                if b % 2 == 0:
                    in1 = fpad[:, off:off + nrows * FP]
                else:
                    in1 = fodd[:, off - 1:off - 1 + nrows * FP]
                nc.vector.tensor_mul(pt, in0, in1)
                prodtiles[g][kk] = pt

        # ---- stage 2b: regressor GEMM, column-tiled 128x64 pairs ----
        # The two PE column tiles each process a WHOLE chunk: tile (0,0)
        # accumulates chunk 2g's 25 taps into psum[0:64] while tile (0,64)
        # concurrently accumulates chunk 2g+1's into psum[64:128] (each
        # tile streams its own product views).  25 slots per chunk pair,
        # each chunk's full sum in a single PSUM half -> the finish is one
        # ScalarE bias-copy per chunk, and nothing ever waits on the DVE
        # FIFO (which is deep in product maps until late in stage 2).
        for g in range(NGRP):
            ptiles = prodtiles[g]
            psum2l = ps2a.tile([128, NPX], F32, name="psum2l",
                               tag="psum2l")
            psum2h = ps2b.tile([128, NPX], F32, name="psum2h",
                               tag="psum2h")
            halves = [psum2l[0:64, :], psum2h[64:128, :]]
            k = 0
            for kk, (a, b) in enumerate(SYM):
                pr = ptiles[kk].rearrange("p (r c) -> p r c", c=FP)
                taps = ([(a, b)] if (a, b) == (0, 0)
                        else [(a, b), (-a, -b)])
                for (p, q) in taps:
                    tidx = (p + 2) * 5 + (q + 2)
                    lhsT = wreg_sb[:, tidx * 64:(tidx + 1) * 64]
                    for half in range(2):
                        p8 = half * CROWS
                        if kk == 0:
                            rhs = pr[:, p8:p8 + CROWS, 2:2 + W]
                        elif (p, q) == (a, b):
                            rhs = pr[:, p8 + a:p8 + a + CROWS, 2:2 + W]
                        else:
                            rhs = pr[:, p8:p8 + CROWS, 2 - b:2 - b + W]
                        nc.tensor.matmul(halves[half], lhsT, rhs,
                                         start=(k == 0), stop=(k == 24),
                                         tile_position=(0, 64 * half))
                    k += 1

            for half, ps in ((0, psum2l[0:64, :]), (1, psum2h[64:128, :])):
                outt = outp.tile([COUT, NPX], F32, name="outsb",
                                 tag="outsb", bufs=4)
                nc.scalar.activation(outt, ps, AF.Identity,
                                     bias=breg_sb, scale=1.0)
                i = 2 * g + half
                nc.sync.dma_start(out=out[:, i * NPX:(i + 1) * NPX],
                                  in_=outt)


def build_nc():
    nc = bacc.Bacc("TRN2", target_bir_lowering=False, debug=False,
                   num_devices=NCORES)
    x = nc.dram_tensor("x", [CIN, HW], BF16, kind="ExternalInput").ap()
    wext = nc.dram_tensor("wext", [128, 18 * 128], BF16,
                          kind="ExternalInput").ap()
    wreg = nc.dram_tensor("wreg", [128, 25 * 64], BF16,
                          kind="ExternalInput").ap()
    bext = nc.dram_tensor("bext", [128, 1], F32, kind="ExternalInput").ap()
    breg = nc.dram_tensor("breg", [64, 1], F32, kind="ExternalInput").ap()
    out = nc.dram_tensor("out", [COUT, HW], F32, kind="ExternalOutput").ap()
    with tile.TileContext(nc) as tc:
        build_body(nc, tc, x, wext, wreg, bext, breg, out)
    nc.compile()
    return nc


def prep_in_maps(x, w_ext, b_ext, w_reg, b_reg):
    BFH = ml_dtypes.bfloat16
    x = np.ascontiguousarray(np.asarray(x, dtype=np.float32))
    w_ext = np.asarray(w_ext, dtype=np.float32)
    w_reg = np.asarray(w_reg, dtype=np.float32)
    b_ext = np.asarray(b_ext, dtype=np.float32)
    b_reg = np.asarray(b_reg, dtype=np.float32)

    # lhsT layouts: wext [cin(128-part), (cintile,tap)*cc], wreg [cc, tap*cout]
    w1 = np.transpose(w_ext, (1, 2, 3, 0))          # [CIN, 3, 3, CC]
    wext_p = np.zeros((128, 18, 128), np.float32)
    for t in range(2):
        for du in range(3):
            for dv in range(3):
                wext_p[:, t * 9 + du * 3 + dv, :] = \
                    w1[t * 128:(t + 1) * 128, du, dv, :]
    wext_p = np.ascontiguousarray(wext_p.reshape(128, 18 * 128).astype(BFH))
    w2 = np.transpose(w_reg, (1, 2, 3, 0))          # [CC, 5, 5, COUT]
    wreg_p = np.ascontiguousarray(w2.reshape(128, 25 * 64).astype(BFH))
    bext_p = np.ascontiguousarray(b_ext.reshape(128, 1))
    breg_p = np.ascontiguousarray(b_reg.reshape(64, 1))
    xb = np.ascontiguousarray(x.reshape(B, CIN, HW).astype(BFH))

    return [{
        "x": xb[b],
        "wext": wext_p,
        "wreg": wreg_p,
        "bext": bext_p,
        "breg": breg_p,
    } for b in range(B)]


_NC_CACHE = None


def kernel(x, w_ext, b_ext, w_reg, b_reg):
    global _NC_CACHE
    if _NC_CACHE is None:
        _NC_CACHE = build_nc()
    nc = _NC_CACHE
    in_maps = prep_in_maps(x, w_ext, b_ext, w_reg, b_reg)
    res = run_bass_kernel_spmd(nc, in_maps, list(range(NCORES)))
    return np.stack([res.results[b]["out"].reshape(COUT, H, W)
                     for b in range(B)], axis=0)
